# revision 36
# baseline (speedup 1.0000x reference)
"""Trainium2 Bass kernel for the PCNN recurrence (nn_CCNN1d).

Model (per sample, recurrence over T steps, state vectors of length L):
    f = df*f + x_t + conv3(y, w)          # learned 3-tap conv, zero pad
    l = dl*l + (y shifted left + right)   # fixed [1,0,1] kernel
    u = f * (1 + 0.5*l)
    e = de*e + 10*y
    y = sigmoid(u - e)
outputs y per step.

Sharding: data-parallel over batch B=32 -> 4 samples per NeuronCore x 8.

Per-core layout ("fine-L"): L=8192 split into 64 blocks of 128; partition
p = position within block.  y lives in a [128 x 264] tile whose data
window is columns [2:262) = 4 sample groups of 65 (64 data blocks + 1
zero pad column); column 1 is the zero left-halo source of group 0.  The
3-tap conv along L becomes one banded 128x128 stationary matmul
(within-block taps) plus two single-element "halo" stationaries applied
to rhs views shifted by one column (cross-block taps); the zero pad
columns make sample boundaries behave like zero padding.

Shipping mode "v3": the recurrence dynamics are chaotic in max-norm
(per-step perturbations amplify ~2x/step until saturation; measured
final_err ~= per-step injection x ~1800), so the arithmetic must match
the fp32 reference to ~1e-6 per step.  That forces the full tf32-split
conv (Wh@yh + Wl@yh + Wh@yl per tap matrix, yl = y - round_tf32(y)):

    PE   : Pf = I@x [fp32] + {Wc,Wc_l,Hdn,Hdn_l,Hup,Hup_l}@yh
                 + {Wc,Hdn,Hup}@yl           (fp32r, 108ns each)
           Pl = Ddl@l2 [fp32] + {Wl05,Hdn05,Hup05}@{yh,yl}
    DVE  : e2 = de*e2 + y;  f = df*f + Pf;  u = (Pl+1)*f;  v = -10*e2+u
           yl = y - yh
    ACT  : l2 = copy(Pl) [fp32];  yh = sigmoid(v) [fp32r];  y = sigmoid(v)

Cheaper variants (v2*, lag-compensated taps, single-sigmoid) are kept
for reference; all fail the 2e-2 gate at T=64 because their 1e-5..1e-3
per-step rounding injections amplify to 5e-2..1.0.

v3 gains over the original split kernel (identical per-step arithmetic,
so correctness is inherited exactly; only the final step's output is
the tf32-rounded sigmoid, a bounded 5e-4 with no feedback path):
step-0 zero-rhs taps skipped, x preloaded via 64 up-front DMAs,
stationary loads split across the SP/GPSIMD DGE queues, state memsets
on GPSIMD ahead of the Pool DMA issues, stationary fp32r casts deferred
past step 0's DVE chain (depth-4 wait-queue bypass), and the last
step's sigma32/yl/l2 work dropped.  232442ns vs 237421ns (TimelineSim);
steady state 3479ns/step is latency-bound on the serial loop
sigmoid -> taps -> f -> u -> v, with every inter-op gap accounted for
by engine pipeline tails and the 100ns semaphore propagation delay.
"""

import numpy as np

B, T, L = 32, 64, 8192
N_CORES = 8
BPC = B // N_CORES          # samples per core
NBLK = L // 128             # 64 blocks per sample
GW = NBLK + 1               # sample group width incl. 1 pad col
DO = 2                      # data window offset (8-byte aligned)
DW = BPC * GW               # data window width = 260
TW = DO + DW + 2            # tile width = 264
ALPHA_F, ALPHA_L, ALPHA_E, V_E = 0.1, 1.0, 1.0, 10.0

_CACHE = {}


def _round_tf32(a):
    a = np.asarray(a, np.float32)
    ai = a.view(np.int32).astype(np.int64)
    return (((ai + 0x1000) & ~0x1FFF).astype(np.int32)).view(np.float32).reshape(a.shape)


def _patch_tile_drain():
    """This toolchain's walrus allows at most one sync wait per instruction;
    spread the TileContext final-drain waits over single-wait nops."""
    import concourse.tile as tile
    from concourse.vector_clock import ScopedClock

    if getattr(tile.TileContext, "_drain_patched", False):
        return

    def _drain_and_barrier(self, tick_clock, wait_clock):
        nc = self.nc
        probe = nc.sync.nop()
        wait_clock.add_sem_waits(probe.ins, ScopedClock({None: tick_clock.global_clock}))
        si = probe.ins.sync_info
        waits = list(si.on_wait) if si and si.on_wait else []
        if len(waits) > 1:
            si.on_wait = waits[:1]
            for w in waits[1:]:
                extra = nc.sync.nop()
                esi = extra.ins.sync_info
                if esi is None:
                    from concourse import mybir
                    extra.ins.sync_info = mybir.SyncInfo(on_wait=[w], on_update=[])
                else:
                    esi.on_wait = [w]
        nc.sync.drain()
        nc.all_engine_barrier()
        assert self.sems is not None
        popped = nc._tile_sem_poison_stack.pop()
        assert popped is self._sem_poison
        nc.clear_and_free_semaphores(list(self.sems.allocated().values()))
        nc.all_engine_barrier()

    tile.TileContext._drain_and_barrier = _drain_and_barrier
    tile.TileContext._drain_patched = True


def _split_sync_waits(nc):
    """Hoist extra sync waits (>1 per instruction) onto same-engine nops
    inserted right before the instruction."""
    from concourse import mybir

    ctr = 0
    for f in nc.m.functions:
        for bb in f.blocks:
            insts = list(bb.instructions)
            if not any(i.sync_info and i.sync_info.on_wait
                       and len(i.sync_info.on_wait) > 1 for i in insts):
                continue
            new_insts = []
            for inst in insts:
                si = inst.sync_info
                waits = list(si.on_wait) if si and si.on_wait else []
                if len(waits) > 1:
                    for w in waits[:-1]:
                        nop = mybir.InstNoOp(name=f"I-wsplit{ctr}", ins=[],
                                             outs=[])
                        ctr += 1
                        nop.engine = inst.engine
                        nop.sync_info = mybir.SyncInfo(on_wait=[w],
                                                       on_update=[])
                        new_insts.append(nop)
                    si.on_wait = [waits[-1]]
                new_insts.append(inst)
            try:
                bb.instructions[:] = new_insts
            except TypeError:
                bb.instructions = new_insts


def _build_program(n_steps, conv_mode):
    """Build the Bass module. Returns (nc, input_names)."""
    assert conv_mode in ("v2", "v2y32", "v2lsplit", "v2split",
                         "v2lag", "v2lag2", "v3")
    lag = conv_mode in ("v2lag", "v2lag2")
    dual_sig = conv_mode != "v2"       # 2nd fp32 sigmoid for e2 + output
    l32 = conv_mode in ("v2lsplit", "v2split", "v3") or lag
    plsplit = conv_mode in ("v2lsplit", "v2split", "v3") or lag
    pfsplit = conv_mode in ("v2split", "v3")  # yl corr on Pf taps (current)
    # v2lag:  Pf += W_l@yh (current) and df-scaled W@yl two steps back
    # v2lag2: both the W_l residual and the yl taps ride the df-scaled lag
    wl_cur = conv_mode in ("v2lag", "v3")
    wl_lag = conv_mode == "v2lag2"
    ylsplit = plsplit or pfsplit
    _patch_tile_drain()
    from contextlib import ExitStack
    import concourse.bass as bass
    import concourse.tile as tile
    from concourse import mybir

    dt = mybir.dt
    AF = mybir.ActivationFunctionType
    OP = mybir.AluOpType
    df = float(np.float32(np.exp(-ALPHA_F)))
    de = float(np.float32(np.exp(-ALPHA_E)))

    nc = bass.Bass("TRN2", target_bir_lowering=False, debug=False,
                   num_devices=N_CORES)

    xp = nc.dram_tensor("xp", [n_steps, 128, TW], dt.float32,
                        kind="ExternalInput").ap()
    stat_names = ["Ddl", "Ident", "Wc", "Hdn", "Hup", "Wl05", "Hdn05", "Hup05"]
    if conv_mode == "v2lag":
        stat_names += ["Wc_l", "Hdn_l", "Hup_l", "Wcd", "Hdnd", "Hupd"]
    elif conv_mode == "v3":
        stat_names += ["Wc_l", "Hdn_l", "Hup_l"]
    elif conv_mode == "v2lag2":
        stat_names += ["Wcd", "Hdnd", "Hupd", "Wld_c", "Hdnld", "Hupld"]
    stats_dram = {n: nc.dram_tensor(n, [128, 128], dt.float32,
                                    kind="ExternalInput").ap()
                  for n in stat_names}
    yp = nc.dram_tensor("yp", [n_steps, 128, TW], dt.float32,
                        kind="ExternalOutput").ap()

    W = slice(DO, DO + DW)           # data window [2:262)
    WL = slice(DO - 1, DO + DW - 1)  # rhs shifted left  [1:261)
    WR = slice(DO + 1, DO + DW + 1)  # rhs shifted right [3:263)
    XBUFS = 16

    with tile.TileContext(nc) as tc:
        with ExitStack() as ctx:
            const = ctx.enter_context(tc.tile_pool(name="const", bufs=1))
            state = ctx.enter_context(tc.tile_pool(name="state", bufs=2))
            ybufs = ctx.enter_context(tc.tile_pool(name="ybufs", bufs=3))
            xbufs = ctx.enter_context(tc.tile_pool(name="xbufs", bufs=XBUFS))
            tmp = ctx.enter_context(tc.tile_pool(name="tmp", bufs=2))
            psum = ctx.enter_context(tc.tile_pool(name="psum", bufs=2,
                                                  space="PSUM"))

            # first x tiles DMA'd before the stationaries: step 0 only
            # needs x0 + Ident/Ddl (its y-taps read zero tiles and are
            # skipped), so the pipeline starts ~6us earlier
            xts = []
            for t in range(2):
                xt = xbufs.tile([128, TW], dt.float32, tag="x", name=f"x{t}")
                nc.sync.dma_start(xt[:], xp[t])
                xts.append(xt)

            # state tiles; memsets go on the gpsimd queue BEFORE its stat
            # DMA issues (they gate step 0's Ddl matmul and DVE chain)
            l2dt = dt.float32 if l32 else dt.float32r
            f = state.tile([128, DW], dt.float32, tag="f")
            e2 = state.tile([128, DW], dt.float32, tag="e2")
            l2r = state.tile([128, DW], l2dt, tag="l2r")
            nc.gpsimd.memset(f[:], 0.0)
            nc.gpsimd.memset(e2[:], 1.0)
            nc.gpsimd.memset(l2r[:].bitcast(dt.float32), 0.0)
            # stationaries; the 6 tap matrices get an on-chip fp32r rounding
            # copy (the BIR verifier requires fp32r matmul operands to come
            # from a rounding producer); Ident/Ddl run as plain fp32 matmuls.
            # Split the loads across the ACT and SP DGE queues so they land
            # in parallel with the x loads (one queue serializes at ~650ns
            # per dma_start, which stalls steps 1-2 on the casts otherwise).
            stats = {}
            gps_q = stat_names[len(stat_names) // 2:]
            for n in stat_names:
                st = const.tile([128, 128], dt.float32, tag=f"st_{n}")
                eng = nc.gpsimd if n in gps_q else nc.sync
                eng.dma_start(st[:], stats_dram[n][:])
                stats[n] = st
            stats_r = {}
            cast_names = ["Wc", "Hdn", "Hup", "Wl05", "Hdn05", "Hup05"]
            if wl_cur:
                cast_names += ["Wc_l", "Hdn_l", "Hup_l"]
            if lag:
                cast_names += ["Wcd", "Hdnd", "Hupd"]
            if wl_lag:
                cast_names += ["Wld_c", "Hdnld", "Hupld"]
            if not l32:
                cast_names.append("Ddl")

            def emit_casts():
                # deferred until after step 0's DVE ops: the casts wait on
                # late stationary DMAs and would block the in-order DVE
                # queue (wait-queue bypass depth is only 4)
                for n in cast_names:
                    sr = const.tile([128, 128], dt.float32r, tag=f"str_{n}")
                    nc.vector.tensor_copy(sr[:], stats[n][:])
                    stats_r[n] = sr

            def S(n):
                return stats_r[n][:] if n in stats_r else stats[n][:]

            # y buffers: compact states above; y_r keeps halo cols
            # pre-zero the rotating y_r buffers once (strided sigmoid never
            # writes the pad columns, so pads stay zero forever)
            for pz in range(2):
                ypz = ybufs.tile([128, TW], dt.float32r, tag="yr",
                                 name=f"yrpz{pz}")
                nc.vector.memset(ypz[:].bitcast(dt.float32), 0.0)
            yr = ybufs.tile([128, TW], dt.float32r, tag="yr")
            nc.vector.memset(yr[:].bitcast(dt.float32), 0.0)
            if dual_sig:
                for pz in range(2):
                    ypz = ybufs.tile([128, TW], dt.float32, tag="y32",
                                     name=f"y32pz{pz}")
                    nc.vector.memset(ypz[:], 0.0)
                y32 = ybufs.tile([128, TW], dt.float32, tag="y32")
                nc.vector.memset(y32[:], 0.0)
            if ylsplit:
                yl = ybufs.tile([128, TW], dt.float32r, tag="yl")
                nc.vector.memset(yl[:].bitcast(dt.float32), 0.0)
            if lag:
                yl_m1 = ybufs.tile([128, TW], dt.float32r, tag="yl",
                                   name="yl_m1")
                nc.vector.memset(yl_m1[:].bitcast(dt.float32), 0.0)
                yl, yl_m1 = yl_m1, yl   # yl = newest (t-1), yl_m1 = (t-2)
            yr_m1 = yr
            # preload the rest of x: all input DMAs up-front (16 bufs)
            for t in range(2, n_steps):
                xt = xbufs.tile([128, TW], dt.float32, tag="x",
                                name=f"x{t}")
                nc.sync.dma_start(xt[:], xp[t])
                xts.append(xt)

            mm = nc.tensor.matmul
            stt = nc.vector.scalar_tensor_tensor

            for t in range(n_steps):
                if t == 1:
                    emit_casts()
                xt = xts[t]
                Pf = psum.tile([128, DW], dt.float32, tag="Pf")
                Pl = psum.tile([128, DW], dt.float32, tag="Pl")

                # ---- PE: Pf = x + conv3(y, w) ----
                # step 0: y/yl tiles are all-zero; the taps would add exact
                # zeros, so emit only the x/l2 matmuls (bit-identical)
                if t == 0:
                    mm(Pl[:], S("Ddl"), l2r[:], start=True, stop=True)
                    mm(Pf[:], S("Ident"), xt[:, W], start=True, stop=True)
                # no-y / lagged taps first (they run in the sigmoid shadow)
                if t > 0:
                    mm(Pf[:], S("Ident"), xt[:, W], start=True, stop=False)
                if t > 0:
                    if lag:
                        mm(Pf[:], S("Wcd"), yl_m1[:, W], start=False, stop=False)
                        mm(Pf[:], S("Hdnd"), yl_m1[:, WL], start=False, stop=False)
                        mm(Pf[:], S("Hupd"), yl_m1[:, WR], start=False, stop=False)
                    if wl_lag:
                        mm(Pf[:], S("Wld_c"), yr_m1[:, W], start=False, stop=False)
                        mm(Pf[:], S("Hdnld"), yr_m1[:, WL], start=False, stop=False)
                        mm(Pf[:], S("Hupld"), yr_m1[:, WR], start=False, stop=False)
                    mm(Pl[:], S("Ddl"), l2r[:], start=True, stop=False)
                    # y-dependent taps: Pf group first (f_new unblocks earlier)
                    mm(Pf[:], S("Wc"), yr[:, W], start=False, stop=False)
                    if wl_cur:
                        mm(Pf[:], S("Wc_l"), yr[:, W], start=False, stop=False)
                    mm(Pf[:], S("Hdn"), yr[:, WL], start=False, stop=False)
                    if wl_cur:
                        mm(Pf[:], S("Hdn_l"), yr[:, WL], start=False, stop=False)
                    mm(Pf[:], S("Hup"), yr[:, WR], start=False,
                       stop=not (pfsplit or wl_cur))
                    if wl_cur:
                        mm(Pf[:], S("Hup_l"), yr[:, WR], start=False,
                           stop=not pfsplit)
                    if pfsplit:
                        mm(Pf[:], S("Wc"), yl[:, W], start=False, stop=False)
                        mm(Pf[:], S("Hdn"), yl[:, WL], start=False, stop=False)
                        mm(Pf[:], S("Hup"), yl[:, WR], start=False, stop=True)
                    mm(Pl[:], S("Wl05"), yr[:, W], start=False, stop=False)
                    mm(Pl[:], S("Hdn05"), yr[:, WL], start=False, stop=False)
                    mm(Pl[:], S("Hup05"), yr[:, WR], start=False,
                       stop=not plsplit)
                    if plsplit:
                        mm(Pl[:], S("Wl05"), yl[:, W], start=False, stop=False)
                        mm(Pl[:], S("Hdn05"), yl[:, WL], start=False, stop=False)
                        mm(Pl[:], S("Hup05"), yl[:, WR], start=False, stop=True)

                # ---- DVE chain ----
                ysrc = y32[:, W] if dual_sig else yr[:, W].bitcast(dt.float32)
                e2_new = state.tile([128, DW], dt.float32, tag="e2")
                stt(e2_new[:], e2[:], de, ysrc, OP.mult, OP.add)
                f_new = state.tile([128, DW], dt.float32, tag="f")
                stt(f_new[:], f[:], df, Pf[:], OP.mult, OP.add)
                u = tmp.tile([128, DW], dt.float32, tag="u")
                stt(u[:], Pl[:], 1.0, f_new[:], OP.add, OP.mult)
                v = tmp.tile([128, DW], dt.float32, tag="v")
                stt(v[:], e2_new[:], -V_E, u[:], OP.mult, OP.add)

                # ---- ACT ----
                last = t == n_steps - 1
                l2r_new = l2r
                if not last:
                    l2r_new = state.tile([128, DW], l2dt, tag="l2r")
                    nc.scalar.copy(l2r_new[:], Pl[:])

                yr_new = ybufs.tile([128, TW], dt.float32r, tag="yr")
                v_src = v[:].rearrange("p (s c) -> p s c", c=GW)[:, :, 0:GW - 1]
                yr_dst = (yr_new[:, DO - 1:DO - 1 + DW]
                          .rearrange("p (s c) -> p s c", c=GW)[:, :, 1:])
                nc.scalar.activation(yr_dst, v_src, AF.Sigmoid)
                if dual_sig and last:
                    # final step: no consumers of exact y / yl / l2 remain;
                    # ship the tf32-rounded sigmoid bits (bounded 2.4e-4
                    # output-only rounding, no feedback path)
                    nc.sync.dma_start(yp[t], yr_new[:].bitcast(dt.float32))
                elif dual_sig:
                    y32_new = ybufs.tile([128, TW], dt.float32, tag="y32")
                    y32_dst = (y32_new[:, DO - 1:DO - 1 + DW]
                               .rearrange("p (s c) -> p s c", c=GW)[:, :, 1:])
                    nc.scalar.activation(y32_dst, v_src, AF.Sigmoid)
                    if ylsplit:
                        yl_new = ybufs.tile([128, TW], dt.float32r, tag="yl")
                        nc.vector.tensor_tensor(yl_new[:], y32_new[:],
                                                yr_new[:].bitcast(dt.float32),
                                                OP.subtract)
                        if lag:
                            yl_m1 = yl
                        yl = yl_new
                    nc.sync.dma_start(yp[t], y32_new[:])
                    y32 = y32_new
                else:
                    nc.sync.dma_start(yp[t], yr_new[:].bitcast(dt.float32))

                yr_m1 = yr
                f, e2, l2r, yr = f_new, e2_new, l2r_new, yr_new

    _split_sync_waits(nc)
    in_names = ["xp"] + stat_names
    return nc, in_names


def _make_stationaries(w, conv_mode="v2"):
    """matmul computes out[i,j] = sum_p W[p,i]*rhs[p,j]; stationary[p, i]
    maps contraction partition p -> output partition i.  All returned
    matrices are tf32-rounded on the host (bit-compatible with fp32r)."""
    w0, w1, w2 = [np.float32(v) for v in np.asarray(w, np.float32)]
    i = np.arange(128)
    st = {}

    def banded(a, b, c):
        # out[i] = a*y[i-1] + b*y[i] + c*y[i+1]  (within block)
        Wm = np.zeros((128, 128), np.float32)
        Wm[i, i] = b
        Wm[i[1:] - 1, i[1:]] = a      # W[p=i-1, i] = a
        Wm[i[:-1] + 1, i[:-1]] = c    # W[p=i+1, i] = c
        return Wm

    def halo_dn(val):
        # out[0, j] += val * rhs[127, j]  (rhs = y shifted left one column)
        Wm = np.zeros((128, 128), np.float32)
        Wm[127, 0] = val
        return Wm

    def halo_up(val):
        # out[127, j] += val * rhs[0, j]  (rhs = y shifted right one column)
        Wm = np.zeros((128, 128), np.float32)
        Wm[0, 127] = val
        return Wm

    st["Ident"] = np.eye(128, dtype=np.float32)
    st["Ddl"] = np.eye(128, dtype=np.float32) * np.float32(np.exp(-ALPHA_L))
    # host-pre-round the tap matrices (round-half-up, matching the original
    # split kernel bit-for-bit); the residuals are W - round(W), re-rounded
    for name, mk, vals in [("Wc", banded, (w0, w1, w2)),
                           ("Hdn", halo_dn, (w0,)),
                           ("Hup", halo_up, (w2,))]:
        Wf = mk(*vals)
        Wh = _round_tf32(Wf)
        st[name] = Wh
        st[name + "_raw_l"] = _round_tf32(Wf - Wh)
    st["Wl05"] = banded(0.5, 0.0, 0.5)
    st["Hdn05"] = halo_dn(0.5)
    st["Hup05"] = halo_up(0.5)
    df = np.float32(np.exp(-ALPHA_F))
    if conv_mode in ("v2lag", "v3"):
        for a in ("Wc", "Hdn", "Hup"):
            st[a + "_l"] = st[a + "_raw_l"]
    if conv_mode == "v2lag":
        st["Wcd"] = df * _round_tf32(st["Wc"])
        st["Hdnd"] = df * _round_tf32(st["Hdn"])
        st["Hupd"] = df * _round_tf32(st["Hup"])
    elif conv_mode == "v2lag2":
        st["Wcd"] = df * _round_tf32(st["Wc"])
        st["Hdnd"] = df * _round_tf32(st["Hdn"])
        st["Hupd"] = df * _round_tf32(st["Hup"])
        st["Wld_c"] = df * (st["Wc"] - _round_tf32(st["Wc"]))
        st["Hdnld"] = df * (st["Hdn"] - _round_tf32(st["Hdn"]))
        st["Hupld"] = df * (st["Hup"] - _round_tf32(st["Hup"]))
    return {k: v for k, v in st.items() if not k.endswith("_raw_l")}


def _pack_x(xc):
    """[BPC, T, L] -> [T, 128, TW] fine-L layout, data window at DO.
    Values are tf32-rounded on the host (they feed fp32r matmuls)."""
    T_ = xc.shape[1]
    xr = np.ascontiguousarray(
        xc.reshape(BPC, T_, NBLK, 128).transpose(1, 3, 0, 2))  # [T,128,BPC,NBLK]
    out = np.zeros((T_, 128, TW), np.float32)
    g = out[:, :, DO:DO + DW].reshape(T_, 128, BPC, GW)
    g[:, :, :, :NBLK] = xr
    return out


def _unpack_y(ypk, T_):
    """[T, 128, TW] -> [BPC, T, L]"""
    yr = ypk[:, :, DO:DO + DW].reshape(T_, 128, BPC, GW)[:, :, :, :NBLK]
    return np.ascontiguousarray(yr.transpose(2, 0, 3, 1)).reshape(BPC, T_, L)


def run_steps(x, w, n_steps, conv_mode="v2"):
    """Run the kernel for n_steps (full inputs), return [B, n_steps, L]."""
    from concourse.bass_utils import run_bass_kernel_spmd

    key = (n_steps, conv_mode)
    if key not in _CACHE:
        _CACHE[key] = _build_program(n_steps, conv_mode)
    nc, in_names = _CACHE[key]

    st = _make_stationaries(w, conv_mode)
    x = np.asarray(x, np.float32)
    in_maps = []
    for c in range(N_CORES):
        m = {"xp": _pack_x(x[c * BPC:(c + 1) * BPC, :n_steps])}
        m.update(st)
        in_maps.append(m)
    res = run_bass_kernel_spmd(nc, in_maps, list(range(N_CORES)))
    out = np.empty((B, n_steps, L), np.float32)
    for c in range(N_CORES):
        out[c * BPC:(c + 1) * BPC] = _unpack_y(res.results[c]["yp"], n_steps)
    return out


def kernel(x, w):
    return run_steps(x, w, T, conv_mode="v3")
